# revision 9
# baseline (speedup 1.0000x reference)
"""LocallyConnected1d Bass kernel for 8 TRN2 NeuronCores.

Problem: x [64, 64, 512] f32, weight [1, 64, 64, 504, 9] f32
         out[b, o, l] = sum_{i,k} x[b, i, l+k] * weight[0, o, i, l, k]
L_out = 504 is sharded 8 x 63; inputs are laid out on the host (bf16)
and the full result is gathered and cast back to f32.

Design (per core):
  - Block-diagonal position pairing: positions (2g, 2g+1) of a column
    group share 9 matmuls of 64-deep contraction.  The stationary
    tile_q [128,128] holds x col q in rows 0:64 x cols 0:64 and x col
    q+1 in rows 64:128 x cols 64:128 (zeros off-diagonal; shipping the
    zeros densely beats a strided data-only DMA, whose <512 B runs pay
    a 2x descriptor penalty); the moving rhs stacks w[2g, k=p] over
    w[2g+1, k=p]; one matmul writes the FULL [128, 64] group.  That is
    576 moving columns per two positions instead of 640 -- the exact
    576/128 = 4.5 accumulation passes/position this contraction allows.
  - l=62 (group 31 has no partner) runs the classic 5-pass scheme
    MID-STREAM so the kernel tail only drains one bank.
  - Outputs accumulate across 7 PSUM banks; each bank is drained by one
    bulk cast-copy (f32 -> bf16) the moment it closes, overlapping the
    next bank's matmuls.  Tail: penultimate bank drains via the
    pre-warmed ACT engine, the last bank's [128,64] DVE copy is
    uncontended, and one merged SP DMA ships the final two groups.
  - Input DMAs are split across the three DMA rings (SP / ACT / Pool)
    in PE-consumption order with minimum-size first chunks, so the PE
    starts at the first-input latency floor and runs gap-free.
"""

import numpy as np
import ml_dtypes

B = 64
CI = 64
CO = 64
K = 9
L = 512
L_OUT = 504
N_CORES = 8
LP = L_OUT // N_CORES          # 63 positions per core
HALO = LP + K - 1              # 71 x-columns per core
NGRP = 32                      # column groups (l//2); g31 = l62 alone
NQ = 70                        # block-diag stationary tiles q = 0..69
BANK_G = [(0, 7), (7, 14), (14, 21), (21, 28), (28, 30), (30, 31), (31, 32)]
NBANK = len(BANK_G)

W9CHUNKS = [(0, 1), (1, 3), (3, 6), (6, 10), (10, 14), (14, 18), (18, 22),
            (22, 26), (26, 31)]
XBCHUNKS = [(0, 5), (5, 9), (9, 15), (15, 23), (23, 33), (33, 45), (45, 57),
            (57, 70)]

OUT_COLS = NGRP * CO           # 2048 bf16 cols

SP, ACT, POOL = "sync", "scalar", "gpsimd"
# chunks round-robined across rings in PE-consumption order; ring firsts
# carry the PE-start gates (xb0 + w9_0 on HWDGE, xb1 on Pool)
DMA_PLAN = {
    SP: [("xb", 0), ("w9", 1), ("w62", 0), ("xb", 3), ("w9", 4), ("xb", 6),
         ("w9", 7)],
    ACT: [("w9", 0), ("xb", 2), ("w9", 3), ("xb", 5), ("w9", 6), ("w9", 8)],
    POOL: [("xb", 1), ("x62", 0), ("w9", 2), ("xb", 4), ("w9", 5), ("xb", 7)],
}


def _build_bass():
    import concourse.bass as bass
    import concourse.mybir as mybir
    from concourse.tile import TileContext

    dt = mybir.dt.bfloat16
    nc = bass.Bass()

    xb_d = nc.dram_tensor("xb", [128, NQ * 128], dt, kind="ExternalInput")
    w9_d = nc.dram_tensor("w9", [128, 31 * K * CO], dt, kind="ExternalInput")
    x62_d = nc.dram_tensor("x62", [128, 5 * B], dt, kind="ExternalInput")
    w62_d = nc.dram_tensor("w62", [128, 5 * CO], dt, kind="ExternalInput")
    out_d = nc.dram_tensor("out", [128, OUT_COLS], dt, kind="ExternalOutput")

    with TileContext(nc) as tc:
        with (
            tc.tile_pool(name="xc", bufs=1) as xpool,
            tc.tile_pool(name="wc", bufs=1) as wpool,
            tc.tile_pool(name="ps", bufs=1, space="PSUM") as ppool,
            tc.tile_pool(name="ob", bufs=1) as opool,
        ):
            xbtiles = [xpool.tile([128, (e - s) * 128], dt, name=f"xb{c}")
                       for c, (s, e) in enumerate(XBCHUNKS)]
            w9tiles = [wpool.tile([128, (e - s) * K * CO], dt, name=f"w9{c}")
                       for c, (s, e) in enumerate(W9CHUNKS)]
            x62 = xpool.tile([128, 5 * B], dt, name="x62")
            w62 = wpool.tile([128, 5 * CO], dt, name="w62")

            def issue(ring, kind, i):
                eng = getattr(nc, ring)
                if kind == "w9":
                    s, e = W9CHUNKS[i]
                    eng.dma_start(out=w9tiles[i],
                                  in_=w9_d[:, s * K * CO:e * K * CO])
                elif kind == "xb":
                    s, e = XBCHUNKS[i]
                    eng.dma_start(out=xbtiles[i],
                                  in_=xb_d[:, s * 128:e * 128])
                elif kind == "x62":
                    eng.dma_start(out=x62, in_=x62_d[:, :])
                elif kind == "w62":
                    eng.dma_start(out=w62, in_=w62_d[:, :])

            maxlen = max(len(v) for v in DMA_PLAN.values())
            for j in range(maxlen):
                for ring in (POOL, SP, ACT):
                    if j < len(DMA_PLAN[ring]):
                        issue(ring, *DMA_PLAN[ring][j])

            out_sb = opool.tile([128, OUT_COLS], dt)
            psum = [ppool.tile([128, 512], mybir.dt.float32,
                               name=f"pb{t}") for t in range(NBANK)]
            # l=63 does not exist; the final [64,64] copy leaves the bottom
            # half of the g31 strip unwritten -- zero it up front.
            nc.vector.memset(out_sb[64:128, 31 * CO:32 * CO], 0.0)
            # pre-warm ACT's lazy activation-table load off the tail path
            scratch = opool.tile([64, 1], mybir.dt.float32, name="preld")
            nc.scalar.copy(out=scratch, in_=w62[0:64, 0:1])

            def chunk_of(v, chunks):
                for c, (s, e) in enumerate(chunks):
                    if s <= v < e:
                        return c, s
                raise AssertionError

            def drain(bank):
                gs, ge = BANK_G[bank]
                lo, hi = gs * CO, ge * CO
                if bank == 4:
                    # penultimate bank on the pre-warmed ACT engine, with
                    # its own ACT out-DMA -- keeps DVE free for bank 5
                    nc.scalar.copy(out=out_sb[:, lo:hi],
                                   in_=psum[bank][:, :hi - lo])
                    nc.scalar.dma_start(out=out_d[:, lo:hi],
                                        in_=out_sb[:, lo:hi])
                elif bank == 5:
                    # last-closing bank: DVE copy (uncontended), then the
                    # merged final SP DMA ships g30+g31 (g31 drained early)
                    nc.vector.tensor_copy(
                        out=out_sb[:, lo:hi], in_=psum[bank][:, :hi - lo])
                    nc.sync.dma_start(out=out_d[:, 30 * CO:32 * CO],
                                      in_=out_sb[:, 30 * CO:32 * CO])
                elif bank == 6:
                    nc.vector.tensor_copy(
                        out=out_sb[0:64, lo:hi],
                        in_=psum[bank][0:64, :hi - lo])
                else:
                    nc.vector.tensor_copy(
                        out=out_sb[:, lo:hi], in_=psum[bank][:, :hi - lo])
                    ring = (nc.gpsimd, nc.gpsimd, nc.gpsimd, nc.gpsimd)[bank]
                    ring.dma_start(out=out_d[:, lo:hi],
                                   in_=out_sb[:, lo:hi])

            for g in range(31):
                bank, bs = chunk_of(g, BANK_G)
                slot = g - bs
                outp = psum[bank][:, slot * CO:(slot + 1) * CO]
                wc, wcs = chunk_of(g, W9CHUNKS)
                for p in range(K):
                    q = 2 * g + p
                    xc, xcs = chunk_of(q, XBCHUNKS)
                    lhsT = xbtiles[xc][:, (q - xcs) * 128:(q - xcs + 1) * 128]
                    off = ((g - wcs) * K + p) * CO
                    rhs = w9tiles[wc][:, off:off + CO]
                    nc.tensor.matmul(outp, lhsT, rhs,
                                     start=(p == 0), stop=(p == K - 1))
                if g == BANK_G[bank][1] - 1:
                    drain(bank)
                if g == 12:
                    # l = 62 (lone position of group 31, bank 6) runs
                    # mid-stream so the kernel tail only drains bank 5;
                    # classic 5-pass scheme, copy-only drain
                    outp62 = psum[6][0:64, 0:64]
                    for s in range(4):
                        nc.tensor.matmul(outp62, x62[:, s * B:(s + 1) * B],
                                         w62[:, s * CO:(s + 1) * CO],
                                         start=(s == 0), stop=False)
                    nc.tensor.matmul(outp62, x62[0:64, 4 * B:5 * B],
                                     w62[0:64, 4 * CO:5 * CO],
                                     start=False, stop=True)
                    drain(6)
    _split_multi_waits(nc, mybir)
    return nc


def _split_multi_waits(nc, mybir):
    """This walrus build encodes at most ONE sync wait per instruction;
    hoist extra waits onto single-wait NoOps (semantically identical)."""
    for f in nc.m.functions:
        for bb in f.blocks:
            out = []
            for inst in bb.instructions:
                si = inst.sync_info
                waits = list(si.on_wait) if si is not None and si.on_wait else []
                if len(waits) > 1:
                    for k, w in enumerate(waits[:-1]):
                        out.append(mybir.InstNoOp(
                            name=f"{inst.name}-wsplit{k}",
                            engine=inst.engine,
                            sync_info=mybir.SyncInfo(on_wait=[w], on_update=[]),
                            bass_nofuse=True))
                    inst.sync_info = mybir.SyncInfo(
                        on_wait=[waits[-1]],
                        on_update=list(si.on_update) if si.on_update else [])
                out.append(inst)
            bb.instructions = out


def _prep_inputs(x, weight):
    """Returns list of 8 per-core input dicts."""
    npdt = ml_dtypes.bfloat16
    x = np.asarray(x, np.float32)
    w0 = np.asarray(weight, np.float32)[0]        # [CO, CI, L_OUT, K]

    wt = np.ascontiguousarray(w0.transpose(2, 3, 1, 0))   # [L_OUT, K, CI, CO]
    xt = np.ascontiguousarray(x.transpose(1, 2, 0)).astype(npdt)  # [CI, L, B]

    in_maps = []
    for m in range(N_CORES):
        hs = LP * m
        xs = xt[:, hs:hs + HALO]                  # [CI, 71, B]
        # block-diagonal stationaries tile_q [128, q, 128]
        xb = np.zeros((128, NQ, 128), npdt)
        xb[0:64, :, 0:64] = xs[:, :NQ]            # x col q      (pos 2g)
        xb[64:128, :, 64:128] = xs[:, 1:NQ + 1]   # x col q+1    (pos 2g+1)
        # w9 rows = half*64+i, col = (g*K+p)*CO+o
        a = wt[hs:hs + 62].reshape(31, 2, K, CI, CO)
        w9 = np.ascontiguousarray(a.transpose(1, 3, 0, 2, 4)) \
            .reshape(128, 31 * K * CO).astype(npdt)
        # l=62 extras in the classic pair layout
        xf = xs.astype(np.float32)                # [CI, 71, B]
        x62 = np.zeros((128, 5 * B), np.float32)
        for s in range(4):
            x62[0:64, s * B:(s + 1) * B] = xf[:, 62 + 2 * s]
            x62[64:128, s * B:(s + 1) * B] = xf[:, 63 + 2 * s]
        x62[0:64, 4 * B:] = xf[:, 70]
        wl = wt[hs + 62]                          # [K, CI, CO]
        w62 = np.zeros((128, 5 * CO), np.float32)
        w62[:, :4 * CO] = (wl[:8].reshape(4, 128, CO)
                           .transpose(1, 0, 2).reshape(128, 4 * CO))
        w62[0:64, 4 * CO:] = wl[8]
        in_maps.append({
            "xb": np.ascontiguousarray(xb).reshape(128, NQ * 128),
            "w9": w9,
            "x62": x62.astype(npdt),
            "w62": w62.astype(npdt),
        })
    return in_maps


def _decode_outputs(results):
    outs = []
    for r in results:
        v = np.asarray(r["out"]).astype(np.float32)
        # [h*64+b, g*64+o] -> out[b, o, l], l = 2g+h
        t = (v.reshape(2, 64, NGRP, CO)
             .transpose(1, 3, 2, 0)
             .reshape(B, CO, NGRP * 2)[:, :, :LP])
        outs.append(t)
    return np.concatenate(outs, axis=2).astype(np.float32)  # [B, CO, L_OUT]


_CACHED_NC = None


def kernel(x, weight):
    global _CACHED_NC
    from concourse.bass_utils import run_bass_kernel_spmd

    if _CACHED_NC is None:
        _CACHED_NC = _build_bass()
    in_maps = _prep_inputs(x, weight)
    res = run_bass_kernel_spmd(_CACHED_NC, in_maps, core_ids=list(range(N_CORES)))
    return _decode_outputs(res.results)
